# revision 17
# baseline (speedup 1.0000x reference)
"""MultiHeadTEAttention TRN2 kernel — 8-core SPMD, one batch element per core.

v3 architecture ("transposed-R single-exp", per core, batch m):
  - Exploits symmetry of relu(a[q,c] + b[k,c]) in q<->k: the R tensor is
    built TRANSPOSED: R~[(qlo,c)-part, k-free] = relu(B_all + a2[:,col]),
    where B_all[(qlo,c), k] = -(tk@kw1)[k,c] (replicated over qlo) and
    a2[(qlo,c), (qt,qg)] = (tq@kw1 + kb1)[q,c] with q = qt*128+qlo*16+qg.
  - Bias contraction with kw2 uses R~ as the matmul STATIONARY:
    out[k-128, (h',qlo)] lands directly in [k, q] layout, ACCUMULATED into
    the token-dots PSUM (dots matmuls first with start=True, then strided
    bias matmuls with start=False).  kb2 is constant over k so it cancels
    in softmax and is dropped.
  - ONE exp per score element (ACT), emitting P^T[k, q] bf16 directly —
    no transpose, no exp(bias)*exp(dots) multiply, no shuffle DMAs.
  - kT is stored zero-padded per head parity (kTe/kTo) so every dots
    matmul is a full 128-row stationary at tile_position (0,0) — 64-row
    stationaries based at partition 64 fail to load on this runtime.
  - AV: lhsT = P^T tile, rhs = v5[k, h, 64|ones] -> out [q, 64|denom];
    normalized via reciprocal of the ones-column (recip on DVE, scale on
    GpSimd).
  - x/attn transposes via DmaTransposeAnt (xbar; out[p,j,c] = in[c, j*128+p]).
  - Output projection: lhsT = attnT j-block, rhs = wout j-block -> [q, dx].
  - Pipeline (pitch ~19.6us/chunk): scores-hg0 -> AV-hg0 overlaps
    scores-hg1 -> AV-hg1; tails run during the NEXT chunk's scores; R~
    slabs for chunk qc+1 are produced on DVE during chunk qc.
"""

import contextlib

import numpy as np

import concourse.bass as bass
import concourse.mybir as mybir
import concourse.tile as tile
from concourse import bacc, bass_utils

F32 = mybir.dt.float32
BF16 = mybir.dt.bfloat16
AX = mybir.AluOpType
AF = mybir.ActivationFunctionType

M, NQ, NKV, DX, DT = 8, 1024, 1024, 512, 2
H, HD = 8, 64
INNER = H * HD          # 512
KHID = 16               # c
SCALE = HD ** -0.5
P = 128
NKT = NKV // P          # 8 k-tiles
NQT = NQ // P           # 8 q-tiles
QC = 256                # q-chunk (2 q-tiles)
NQC = NQ // QC          # 4
NJ = DX // P            # 4 dx blocks
KH = NKV // 2           # k-half


def build_kernel(nc: bass.Bass):
    d_xq = nc.dram_tensor("xq", [NQ, DX], F32, kind="ExternalInput").ap()
    d_xk = nc.dram_tensor("xk", [NKV, DX], F32, kind="ExternalInput").ap()
    d_xv = nc.dram_tensor("xv", [NKV, DX], F32, kind="ExternalInput").ap()
    d_tq = nc.dram_tensor("tq", [NQ, DT], F32, kind="ExternalInput").ap()
    d_tk = nc.dram_tensor("tk", [NKV, DT], F32, kind="ExternalInput").ap()
    d_wq = nc.dram_tensor("w_q", [DX, INNER], F32, kind="ExternalInput").ap()
    d_wk = nc.dram_tensor("w_k", [DX, INNER], F32, kind="ExternalInput").ap()
    d_wv = nc.dram_tensor("w_v", [DX, INNER], F32, kind="ExternalInput").ap()
    d_wout = nc.dram_tensor("w_out", [INNER, DX], F32, kind="ExternalInput").ap()
    d_bout = nc.dram_tensor("b_out", [DX], F32, kind="ExternalInput").ap()
    d_kw1 = nc.dram_tensor("kw1", [DT, KHID], F32, kind="ExternalInput").ap()
    d_kb1 = nc.dram_tensor("kb1", [KHID], F32, kind="ExternalInput").ap()
    d_kw2 = nc.dram_tensor("kw2", [KHID, H], F32, kind="ExternalInput").ap()
    d_kb2 = nc.dram_tensor("kb2", [H], F32, kind="ExternalInput").ap()
    d_out = nc.dram_tensor("out", [NQ, DX], F32, kind="ExternalOutput").ap()

    with tile.TileContext(nc) as tc:
        _body(tc, d_xq, d_xk, d_xv, d_tq, d_tk, d_wq, d_wk, d_wv, d_wout,
              d_bout, d_kw1, d_kb1, d_kw2, d_out)
    return nc


def _ap(t_ap, offset_elems, dims):
    """Raw AP on the same tensor with explicit [step, num] dims (elems)."""
    return bass.AP(tensor=t_ap.tensor, offset=t_ap.offset + offset_elems,
                   ap=[list(d) for d in dims])


def _fs(t_ap):
    """Free size (elems/partition) of a tile AP = its partition stride."""
    return t_ap.ap[0][0]


def _body(tc, d_xq, d_xk, d_xv, d_tq, d_tk, d_wq, d_wk, d_wv, d_wout,
          d_bout, d_kw1, d_kb1, d_kw2, d_out):
    nc = tc.nc
    ctx = contextlib.ExitStack()
    persist = ctx.enter_context(tc.tile_pool(name="persist", bufs=1))
    dram_pool = ctx.enter_context(tc.tile_pool(name="drsc", bufs=1, space="DRAM"))
    ps_o = ctx.enter_context(tc.tile_pool(name="ps_o", bufs=2, space="PSUM"))

    # ================= phase 0: constants & small precompute =================
    ctx0 = contextlib.ExitStack()
    p0 = ctx0.enter_context(tc.tile_pool(name="p0", bufs=1))
    psum0 = ctx0.enter_context(tc.tile_pool(name="psum0", bufs=2, space="PSUM"))

    tc.tile_set_cur_wait(0.0001)
    tqT = p0.tile([DT, NQ], F32)
    tkT = p0.tile([DT, NKV], F32)
    nc.sync.dma_start(out=tqT[:, :],
                      in_=_ap(d_tq, 0, [[1, DT], [DT, NQ], [1, 1]]))
    nc.sync.dma_start(out=tkT[:, :],
                      in_=_ap(d_tk, 0, [[1, DT], [DT, NKV], [1, 1]]))
    # kw1 replicated over qlo: kw1_rep[t, qlo*16+c] = kw1[t, c]
    kw1_rep = p0.tile([DT, P], F32)
    nc.sync.dma_start(out=kw1_rep[:, :],
                      in_=_ap(d_kw1, 0, [[KHID, DT], [0, 8], [1, KHID]]))
    kw1k_sb = p0.tile([DT, KHID], F32)
    nc.sync.dma_start(out=kw1k_sb[:, :], in_=d_kw1[:, :])
    kb1_sb = p0.tile([KHID, 1], F32)
    nc.sync.dma_start(out=kb1_sb[:, :],
                      in_=bass.AP(tensor=d_kb1.tensor, offset=d_kb1.offset,
                                  ap=[[1, KHID], [1, 1]]))
    kw2_bf = p0.tile([KHID, H], BF16)
    nc.gpsimd.dma_start(out=kw2_bf[:, :], in_=d_kw2[:, :])

    tqT_bf = p0.tile([DT, NQ], BF16)
    tkT_bf = p0.tile([DT, NKV], BF16)
    kw1rn_bf = p0.tile([DT, P], BF16)   # negated replicated kw1
    kw1k_bf = p0.tile([DT, KHID], BF16)
    nc.vector.tensor_copy(tqT_bf[:, :], tqT[:, :])
    nc.vector.tensor_copy(tkT_bf[:, :], tkT[:, :])
    nc.vector.tensor_scalar(out=kw1rn_bf[:, :], in0=kw1_rep[:, :],
                            scalar1=-1.0, scalar2=0.0,
                            op0=AX.mult, op1=AX.bypass)
    nc.vector.tensor_copy(kw1k_bf[:, :], kw1k_sb[:, :])

    # B_all[(qlo,c), k] = -(tk @ kw1)[k, c]  (replicated over qlo)
    B_all = persist.tile([P, NKV], BF16)
    for half in range(2):
        bps = psum0.tile([P, 512], F32, tag="ph0")
        nc.tensor.matmul(bps[:, :], kw1rn_bf[:, :],
                         tkT_bf[:, half * 512:(half + 1) * 512],
                         start=True, stop=True)
        nc.scalar.activation(B_all[:, half * 512:(half + 1) * 512],
                             bps[:, :], AF.Copy)

    # a[c, q] = (tq @ kw1)[q, c] + kb1[c]  ->  DRAM bounce ->
    # a2[(qlo,c), qt*16+qg] = a[c, qt*128+qlo*16+qg]
    a_sb = p0.tile([KHID, NQ], F32)
    for half in range(2):
        aps = psum0.tile([P, 512], F32, tag="ph0")
        nc.tensor.matmul(aps[0:KHID, :], kw1k_bf[:, :],
                         tqT_bf[:, half * 512:(half + 1) * 512],
                         start=True, stop=True)
        nc.scalar.activation(a_sb[:, half * 512:(half + 1) * 512],
                             aps[0:KHID, :], AF.Identity, bias=kb1_sb[:, :])
    a_dram = dram_pool.tile([KHID, NQ], F32)
    nc.sync.dma_start(out=a_dram[:, :], in_=a_sb[:, :])
    a2 = persist.tile([P, P], F32)      # [(qlo,c), (qt,qg)]
    for qlo in range(8):
        nc.sync.dma_start(
            out=a2[qlo * KHID:(qlo + 1) * KHID, :],
            in_=_ap(a_dram, qlo * 16, [[NQ, KHID], [P, NQT], [1, 16]]))

    # E2[hg][(qlo,c), h'*8+qlo'] = delta(qlo=qlo') * kw2[c, hg*4+h']
    E2 = []
    for hg in range(2):
        e2 = persist.tile([P, 32], BF16, name=f"e2_{hg}")
        nc.vector.memset(e2[:, :], 0.0)
        for qlo in range(8):
            nc.scalar.dma_start(
                out=_ap(e2, qlo * 16 * 32 + qlo, [[32, KHID], [8, 4], [1, 1]]),
                in_=_ap(kw2_bf, hg * 4, [[H, KHID], [1, 4], [1, 1]]))
        E2.append(e2)

    bout_bc = persist.tile([P, DX], F32)
    nc.scalar.dma_start(
        out=bout_bc[:, :],
        in_=bass.AP(tensor=d_bout.tensor, offset=d_bout.offset,
                    ap=[[0, P], [1, DX]]))

    # ================= phase 1: loads, transposes, projections ==============
    ctx1 = contextlib.ExitStack()
    p1 = ctx1.enter_context(tc.tile_pool(name="p1", bufs=1))

    def load_w(dram, pool, name, wait):
        w = pool.tile([P, NJ, INNER], BF16, name=name)
        tc.tile_set_cur_wait(wait)
        nc.gpsimd.dma_start(out=w[:, :, :],
                            in_=dram.rearrange("(t p) i -> p t i", p=P))
        return w

    wk_bf = load_w(d_wk, p1, "wk", 0.0002)
    wq_raw = load_w(d_wq, p1, "wq", 0.0004)
    wq_bf = p1.tile([P, NJ, INNER], BF16, name="wqs")
    nc.vector.tensor_scalar(out=wq_bf[:, :, :], in0=wq_raw[:, :, :],
                            scalar1=SCALE, scalar2=0.0,
                            op0=AX.mult, op1=AX.bypass)

    def load_x(dram, name, wait):
        """x loaded in 2 half-DMAs (bf16 cast on gpsimd), transposed per
        128-row tile via xbar into xT[p, j, q] = x^T[j*128+p, q]."""
        xb = p1.tile([P, NQT, DX], BF16, name=name)
        xT = p1.tile([P, NJ, NQ], BF16, name=name + "T")
        for hf in range(2):
            tc.tile_set_cur_wait(wait + 0.0015 * hf)
            nc.gpsimd.dma_start(
                out=xb[:, hf * 4:(hf + 1) * 4, :],
                in_=_ap(dram, hf * 4 * P * DX, [[DX, P], [P * DX, 4], [1, DX]]))
            for tt in range(4):
                t = hf * 4 + tt
                eng = nc.sync if t % 2 == 0 else nc.scalar
                eng.dma_start_transpose(
                    _ap(xT, t * P, [[_fs(xT), P], [NQ, NJ], [1, P]]),
                    xb[:, t, :])
        return xb, xT

    xk_b, xkT = load_x(d_xk, "xk", 0.0006)
    xq_b, xqT = load_x(d_xq, "xq", 0.0040)
    wv_bf = load_w(d_wv, p1, "wv", 0.0062)
    xv_b, xvT = load_x(d_xv, "xv", 0.0066)
    wout_bf = load_w(d_wout, persist, "wout", 0.0095)

    qT_bf = persist.tile([P, NJ, NQ], BF16)
    # k^T stored zero-padded per head parity so every dots matmul is a full
    # 128-row stationary at tile_position (0,0): kTe keeps even-head rows
    # (0:64 of each it-slab), kTo keeps odd-head rows; the other half is 0.
    kTe = persist.tile([P, NJ, NKV], BF16)
    kTo = persist.tile([P, NJ, NKV], BF16)
    v5 = persist.tile([P, NKT, H, 65], BF16)
    fs_v5 = _fs(v5)
    tc.tile_set_cur_wait(0.0012)
    nc.vector.memset(kTe[64:128, :, :], 0.0)
    tc.tile_set_cur_wait(0.010)
    nc.gpsimd.memset(kTo[0:64, :, :], 0.0)

    # k-proj: kT[it*128+r, k] = (xk @ wk)[k, it*128+r]  (copies on ACT)
    for it in range(NJ):
        for jk in range(2):
            tc.tile_set_cur_wait(0.0028 + 0.0008 * (it * 2 + jk))
            pk = ps_o.tile([P, 512], F32, tag="po")
            for j in range(NJ):
                nc.tensor.matmul(pk[:, :], wk_bf[:, j, it * P:(it + 1) * P],
                                 xkT[:, j, jk * 512:(jk + 1) * 512],
                                 start=(j == 0), stop=(j == NJ - 1))
            nc.scalar.activation(kTe[0:64, it, jk * 512:(jk + 1) * 512],
                                 pk[0:64, :], AF.Copy)
            nc.scalar.activation(kTo[64:128, it, jk * 512:(jk + 1) * 512],
                                 pk[64:128, :], AF.Copy)

    # q-proj (copies on ACT)
    for it in range(NJ):
        for jq in range(2):
            tc.tile_set_cur_wait(0.0068 + 0.0008 * (it * 2 + jq))
            pq = ps_o.tile([P, 512], F32, tag="po")
            for j in range(NJ):
                nc.tensor.matmul(pq[:, :], wq_bf[:, j, it * P:(it + 1) * P],
                                 xqT[:, j, jq * 512:(jq + 1) * 512],
                                 start=(j == 0), stop=(j == NJ - 1))
            nc.scalar.activation(qT_bf[:, it, jq * 512:(jq + 1) * 512],
                                 pq[:, :], AF.Copy)

    # v-proj into v5 [k, kt, h, 64|1]  (copies on DVE)
    for kt in range(NKT):
        tc.tile_set_cur_wait(0.0105 + 0.0008 * kt)
        pv = ps_o.tile([P, 512], F32, tag="po")
        for j in range(NJ):
            nc.tensor.matmul(pv[:, :], xvT[:, j, kt * P:(kt + 1) * P],
                             wv_bf[:, j, :],
                             start=(j == 0), stop=(j == NJ - 1))
        nc.vector.tensor_copy(
            _ap(v5, kt * (H * 65), [[fs_v5, P], [65, H], [1, 64]]),
            pv[:, :])
    nc.gpsimd.memset(
        _ap(v5, 64, [[fs_v5, P], [H * 65, NKT], [65, H], [1, 1]]), 1.0)

    # phase 0/1 tiles are fully consumed once projections are emitted
    ctx1.close()
    ctx0.close()

    # ================= phase 2: attention main loop =========================
    ctx2 = contextlib.ExitStack()
    ps_dt = ctx2.enter_context(tc.tile_pool(name="ps_dt", bufs=2, space="PSUM"))
    ps_av = ctx2.enter_context(tc.tile_pool(name="ps_av", bufs=2, space="PSUM"))
    r_pool = ctx2.enter_context(tc.tile_pool(name="rp", bufs=6))
    pt_pool = ctx2.enter_context(tc.tile_pool(name="pt", bufs=3))
    attn_pool = ctx2.enter_context(tc.tile_pool(name="attn", bufs=4))
    at_pool = ctx2.enter_context(tc.tile_pool(name="at", bufs=2))
    o_pool = ctx2.enter_context(tc.tile_pool(name="op", bufs=2))
    zr_pool = ctx2.enter_context(tc.tile_pool(name="zr", bufs=4))

    def produce_r(qt, kh, wait, step=0.00019):
        """R~ half-slab for q-tile qt, k-half kh: [P=(qlo,c), qg, k-512]."""
        slab = r_pool.tile([P, 16, KH], BF16, tag="rslab")
        for qg in range(16):
            tc.tile_set_cur_wait(wait + step * qg)
            nc.vector.tensor_scalar(
                out=slab[:, qg, :], in0=B_all[:, kh * KH:(kh + 1) * KH],
                scalar1=a2[:, qt * 16 + qg: qt * 16 + qg + 1],
                scalar2=0.0, op0=AX.add, op1=AX.max)
        return slab

    BASE = 0.0150
    PITCH = 0.0196

    # chunk-0 slabs produced during phase 1 (DVE free after early converts)
    slabs = [[produce_r(0, 0, 0.0020), produce_r(1, 0, 0.0052)],
             [produce_r(0, 1, 0.0084), produce_r(1, 1, 0.0116)]]

    def scores(qc, hg, qc_slabs, t0):
        """8 score groups (one per kt) for head-group hg; exp -> pt."""
        pt = pt_pool.tile([P, 4, NKT, QC], BF16, tag="pt", name="pt")
        fs_pt = _fs(pt)
        for kt in range(NKT):
            with tc.tile_wait_until(t0 + 0.00102 * kt):
                dt = ps_dt.tile([P, 4, QC], F32, tag="dt")
                fs_dt = _fs(dt)
                for bank in range(2):
                    for hh in range(2):
                        h = hg * 4 + bank * 2 + hh
                        it = h // 2
                        kT_h = kTe if h % 2 == 0 else kTo
                        nc.tensor.matmul(
                            dt[:, bank * 2 + hh, :],
                            kT_h[:, it, kt * P:(kt + 1) * P],
                            qT_bf[:, it, qc * QC:(qc + 1) * QC],
                            start=(hh == 0), stop=False,
                            skip_group_check=True)
                    for qth in range(2):
                        slab = qc_slabs[kt // 4][qth]
                        fs_r = _fs(slab)
                        for qg in range(16):
                            last = (qth == 1 and qg == 15)
                            nc.tensor.matmul(
                                _ap(dt, bank * 2 * QC + qth * P + qg,
                                    [[fs_dt, P], [QC, 2], [16, 8]]),
                                _ap(slab, qg * KH + (kt % 4) * P,
                                    [[fs_r, P], [1, P]]),
                                E2[hg][:, bank * 16:(bank + 1) * 16],
                                start=False, stop=last,
                                skip_group_check=True)
                nc.scalar.activation(
                    _ap(pt, kt * QC,
                        [[fs_pt, P], [NKT * QC, 4], [1, QC]]),
                    dt[:, :, :], AF.Exp)
        return pt

    def av_half(qc, hg, pt, attns, t0):
        """AV + normalize for the 4 heads of hg."""
        fs_pt = _fs(pt)
        for i in range(8):
            hh, qt2 = i // 2, i % 2
            h = hg * 4 + hh
            with tc.tile_wait_until(t0 + 0.0002 * i):
                av = ps_av.tile([P, 65], F32, tag="av")
                fs_av = _fs(av)
                for kt in range(NKT):
                    nc.tensor.matmul(
                        av[:, :],
                        _ap(pt, hh * (NKT * QC) + kt * QC + qt2 * P,
                            [[fs_pt, P], [1, P]]),
                        v5[:, kt, h, :],
                        start=(kt == 0), stop=(kt == NKT - 1))
                zr = zr_pool.tile([P, 1], F32, tag="zr")
                nc.vector.reciprocal(
                    zr[:, :], _ap(av, 64, [[fs_av, P], [1, 1]]))
                nc.vector.tensor_scalar(
                    out=attns[qt2][:, h * 64:(h + 1) * 64],
                    in0=_ap(av, 0, [[fs_av, P], [1, 64]]),
                    scalar1=zr[:, :], scalar2=0.0,
                    op0=AX.mult, op1=AX.bypass)

    def tail(qc, attns, t0):
        for qt2 in range(2):
            with tc.tile_wait_until(t0 + 0.0020 * qt2):
                at = at_pool.tile([P, NJ, P], BF16, tag="at")
                eng = nc.sync if qt2 == 0 else nc.scalar
                eng.dma_start_transpose(at[:, :, :], attns[qt2][:, :])
                po = ps_o.tile([P, DX], F32, tag="po")
                for j in range(NJ):
                    nc.tensor.matmul(po[:, :], at[:, j, :], wout_bf[:, j, :],
                                     start=(j == 0), stop=(j == NJ - 1))
                o_sb = o_pool.tile([P, DX], F32, tag="osb")
                nc.vector.tensor_add(o_sb[:, :], po[:, :], bout_bc[:, :])
                qt_g = qc * 2 + qt2
                nc.sync.dma_start(out=d_out[qt_g * P:(qt_g + 1) * P, :],
                                  in_=o_sb[:, :])

    for qc in range(NQC):
        t0 = BASE + PITCH * qc
        qc_slabs = slabs
        if qc + 1 < NQC:
            # k-half-0 slabs of qc+1 go to fresh ring buffers; k-half-1
            # slabs overwrite qc's k-half-0 slabs (free after hg1/kt3)
            slabs = [[produce_r(2 * qc + 2, 0, t0 + 0.0005),
                      produce_r(2 * qc + 3, 0, t0 + 0.0037)],
                     [produce_r(2 * qc + 2, 1, t0 + 0.0125),
                      produce_r(2 * qc + 3, 1, t0 + 0.0157)]]

        a_t0 = attn_pool.tile([P, INNER], BF16, tag="attn", name="a_t0")
        a_t1 = attn_pool.tile([P, INNER], BF16, tag="attn", name="a_t1")
        attns = [a_t0, a_t1]
        pt0 = scores(qc, 0, qc_slabs, t0)
        av_half(qc, 0, pt0, attns, t0 + 0.0090)
        pt1 = scores(qc, 1, qc_slabs, t0 + 0.0083)
        av_half(qc, 1, pt1, attns, t0 + 0.0173)
        tail(qc, attns, t0 + PITCH + 0.0012)

    ctx2.close()
    ctx.close()


_NC_CACHE = None


def _get_nc():
    global _NC_CACHE
    if _NC_CACHE is None:
        nc = bacc.Bacc("TRN2", target_bir_lowering=False, debug=False,
                       enable_asserts=False, num_devices=M)
        build_kernel(nc)
        nc.compile()
        _NC_CACHE = nc
    return _NC_CACHE


def kernel(**inputs):
    nc = _get_nc()
    # kb2 is declared but unused on-device: constant over the softmax axis,
    # it cancels in the softmax.
    shared = {n: np.ascontiguousarray(np.asarray(inputs[n], dtype=np.float32))
              for n in ["w_q", "w_k", "w_v", "w_out", "b_out",
                        "kw1", "kb1", "kw2", "kb2"]}
    in_maps = []
    for i in range(M):
        m = dict(shared)
        for n in ["xq", "xk", "xv", "tq", "tk"]:
            m[n] = np.ascontiguousarray(np.asarray(inputs[n][i], dtype=np.float32))
        in_maps.append(m)
    res = bass_utils.run_bass_kernel_spmd(nc, in_maps, core_ids=list(range(M)))
    out = np.stack([res.results[i]["out"] for i in range(M)], axis=0)
    return out.astype(np.float32)


if __name__ == "__main__":
    import reference
    inputs = {k: np.asarray(v) for k, v in reference.setup_inputs().items()}
    out = kernel(**inputs)
    print("out", out.shape, out.dtype)


# revision 26
# speedup vs baseline: 1.2052x; 1.2052x over previous
"""MultiHeadTEAttention TRN2 kernel — 8-core SPMD, one batch element per core.

v3 architecture ("transposed-R single-exp", per core, batch m):
  - Exploits symmetry of relu(a[q,c] + b[k,c]) in q<->k: the R tensor is
    built TRANSPOSED: R~[(qlo,c)-part, k-free] = relu(B_all + a2[:,col]),
    where B_all[(qlo,c), k] = -(tk@kw1)[k,c] (replicated over qlo) and
    a2[(qlo,c), (qt,qg)] = (tq@kw1 + kb1)[q,c] with q = qt*128+qlo*16+qg.
  - Bias contraction with kw2 uses R~ as the matmul STATIONARY:
    out[k-128, (h',qlo)] lands directly in [k, q] layout, ACCUMULATED into
    the token-dots PSUM (dots matmuls first with start=True, then strided
    bias matmuls with start=False).  kb2 is constant over k so it cancels
    in softmax and is dropped.
  - ONE exp per score element (ACT), emitting P^T[k, q] bf16 directly —
    no transpose, no exp(bias)*exp(dots) multiply, no shuffle DMAs.
  - kT is stored zero-padded per head parity (kTe/kTo) so every dots
    matmul is a full 128-row stationary at tile_position (0,0) — 64-row
    stationaries based at partition 64 fail to load on this runtime.
  - AV: lhsT = P^T tile, rhs = v5[k, h, 64|ones] -> out [q, 64|denom];
    normalized via reciprocal of the ones-column (recip on DVE, scale on
    GpSimd).
  - x/attn transposes via DmaTransposeAnt (xbar; out[p,j,c] = in[c, j*128+p]).
  - Output projection: lhsT = attnT j-block, rhs = wout j-block -> [q, dx].
  - Pipeline (pitch ~19.6us/chunk): scores-hg0 -> AV-hg0 overlaps
    scores-hg1 -> AV-hg1; tails run during the NEXT chunk's scores; R~
    slabs for chunk qc+1 are produced on DVE during chunk qc.
"""

import contextlib

import numpy as np

import concourse.bass as bass
import concourse.mybir as mybir
import concourse.tile as tile
from concourse import bacc, bass_utils

F32 = mybir.dt.float32
BF16 = mybir.dt.bfloat16
AX = mybir.AluOpType
AF = mybir.ActivationFunctionType

M, NQ, NKV, DX, DT = 8, 1024, 1024, 512, 2
H, HD = 8, 64
INNER = H * HD          # 512
KHID = 16               # c
SCALE = HD ** -0.5
P = 128
NKT = NKV // P          # 8 k-tiles
NQT = NQ // P           # 8 q-tiles
QC = 256                # q-chunk (2 q-tiles)
NQC = NQ // QC          # 4
NJ = DX // P            # 4 dx blocks
KH = NKV // 2           # k-half


def build_kernel(nc: bass.Bass):
    d_xq = nc.dram_tensor("xq", [NQ, DX], F32, kind="ExternalInput").ap()
    d_xk = nc.dram_tensor("xk", [NKV, DX], F32, kind="ExternalInput").ap()
    d_xv = nc.dram_tensor("xv", [NKV, DX], F32, kind="ExternalInput").ap()
    d_tq = nc.dram_tensor("tq", [NQ, DT], F32, kind="ExternalInput").ap()
    d_tk = nc.dram_tensor("tk", [NKV, DT], F32, kind="ExternalInput").ap()
    d_wq = nc.dram_tensor("w_q", [DX, INNER], F32, kind="ExternalInput").ap()
    d_wk = nc.dram_tensor("w_k", [DX, INNER], F32, kind="ExternalInput").ap()
    d_wv = nc.dram_tensor("w_v", [DX, INNER], F32, kind="ExternalInput").ap()
    d_wout = nc.dram_tensor("w_out", [INNER, DX], F32, kind="ExternalInput").ap()
    d_bout = nc.dram_tensor("b_out", [DX], F32, kind="ExternalInput").ap()
    d_kw1 = nc.dram_tensor("kw1", [DT, KHID], F32, kind="ExternalInput").ap()
    d_kb1 = nc.dram_tensor("kb1", [KHID], F32, kind="ExternalInput").ap()
    d_kw2 = nc.dram_tensor("kw2", [KHID, H], F32, kind="ExternalInput").ap()
    d_kb2 = nc.dram_tensor("kb2", [H], F32, kind="ExternalInput").ap()
    d_out = nc.dram_tensor("out", [NQ, DX], F32, kind="ExternalOutput").ap()

    with tile.TileContext(nc) as tc:
        _body(tc, d_xq, d_xk, d_xv, d_tq, d_tk, d_wq, d_wk, d_wv, d_wout,
              d_bout, d_kw1, d_kb1, d_kw2, d_out)
    return nc


def _ap(t_ap, offset_elems, dims):
    """Raw AP on the same tensor with explicit [step, num] dims (elems)."""
    return bass.AP(tensor=t_ap.tensor, offset=t_ap.offset + offset_elems,
                   ap=[list(d) for d in dims])


def _fs(t_ap):
    """Free size (elems/partition) of a tile AP = its partition stride."""
    return t_ap.ap[0][0]


def _body(tc, d_xq, d_xk, d_xv, d_tq, d_tk, d_wq, d_wk, d_wv, d_wout,
          d_bout, d_kw1, d_kb1, d_kw2, d_out):
    nc = tc.nc
    ctx = contextlib.ExitStack()
    persist = ctx.enter_context(tc.tile_pool(name="persist", bufs=1))
    dram_pool = ctx.enter_context(tc.tile_pool(name="drsc", bufs=1, space="DRAM"))
    ps_o = ctx.enter_context(tc.tile_pool(name="ps_o", bufs=2, space="PSUM"))

    # ================= phase 0: constants & small precompute =================
    # NOTE: no pool is ever closed mid-kernel — closing a pool inserts
    # all-engine drain barriers (address reuse) that serialize the pipeline.
    p0 = ctx.enter_context(tc.tile_pool(name="p0", bufs=1))

    tc.tile_set_cur_wait(0.0001)
    tqT = p0.tile([DT, NQ], F32)
    tkT = p0.tile([DT, NKV], F32)
    tqT_bf = p0.tile([DT, NQ], BF16)
    tkT_bf = p0.tile([DT, NKV], BF16)
    nc.sync.dma_start(out=tkT[:, :],
                      in_=_ap(d_tk, 0, [[1, DT], [DT, NKV], [1, 1]]))
    nc.sync.dma_start(out=tqT[:, :],
                      in_=_ap(d_tq, 0, [[1, DT], [DT, NQ], [1, 1]]))
    nc.vector.tensor_copy(tkT_bf[:, :], tkT[:, :])
    nc.vector.tensor_copy(tqT_bf[:, :], tqT[:, :])
    # kw1 replicated over qlo: kw1_rep[t, qlo*16+c] = kw1[t, c]
    kw1_rep = p0.tile([DT, P], F32)
    nc.sync.dma_start(out=kw1_rep[:, :],
                      in_=_ap(d_kw1, 0, [[KHID, DT], [0, 8], [1, KHID]]))
    kw1k_sb = p0.tile([DT, KHID], F32)
    nc.sync.dma_start(out=kw1k_sb[:, :], in_=d_kw1[:, :])
    kb1_sb = p0.tile([KHID, 1], F32)
    nc.sync.dma_start(out=kb1_sb[:, :],
                      in_=bass.AP(tensor=d_kb1.tensor, offset=d_kb1.offset,
                                  ap=[[1, KHID], [1, 1]]))
    kw2_bf = p0.tile([KHID, H], BF16)
    nc.gpsimd.dma_start(out=kw2_bf[:, :], in_=d_kw2[:, :])

    kw1rn_bf = p0.tile([DT, P], BF16)   # negated replicated kw1
    kw1k_bf = p0.tile([DT, KHID], BF16)
    nc.vector.tensor_scalar(out=kw1rn_bf[:, :], in0=kw1_rep[:, :],
                            scalar1=-1.0, scalar2=0.0,
                            op0=AX.mult, op1=AX.bypass)
    nc.vector.tensor_copy(kw1k_bf[:, :], kw1k_sb[:, :])

    # B_all[(qlo,c), k] = -(tk @ kw1)[k, c]  (replicated over qlo)
    B_all = persist.tile([P, NKV], BF16)
    for half in range(2):
        bps = ps_o.tile([P, 512], F32, tag="po")
        nc.tensor.matmul(bps[:, :], kw1rn_bf[:, :],
                         tkT_bf[:, half * 512:(half + 1) * 512],
                         start=True, stop=True)
        nc.scalar.activation(B_all[:, half * 512:(half + 1) * 512],
                             bps[:, :], AF.Copy)

    # a[c, q] = (tq @ kw1)[q, c] + kb1[c]  ->  DRAM bounce ->
    # a2[(qlo,c), qt*16+qg] = a[c, qt*128+qlo*16+qg]
    a_sb = p0.tile([KHID, NQ], F32)
    for half in range(2):
        aps = ps_o.tile([P, 512], F32, tag="po")
        nc.tensor.matmul(aps[0:KHID, :], kw1k_bf[:, :],
                         tqT_bf[:, half * 512:(half + 1) * 512],
                         start=True, stop=True)
        nc.scalar.activation(a_sb[:, half * 512:(half + 1) * 512],
                             aps[0:KHID, :], AF.Identity, bias=kb1_sb[:, :])
    a_dram = dram_pool.tile([KHID, NQ], F32)
    nc.sync.dma_start(out=a_dram[:, :], in_=a_sb[:, :])
    a2 = persist.tile([P, P], F32)      # [(qlo,c), (qt,qg)]
    for qlo in range(8):
        nc.sync.dma_start(
            out=a2[qlo * KHID:(qlo + 1) * KHID, :],
            in_=_ap(a_dram, qlo * 16, [[NQ, KHID], [P, NQT], [1, 16]]))

    # E2[hg][(qlo,c), h'*8+qlo'] = delta(qlo=qlo') * kw2[c, hg*4+h']
    E2 = []
    for hg in range(2):
        e2 = persist.tile([P, 32], BF16, name=f"e2_{hg}")
        nc.vector.memset(e2[:, :], 0.0)
        for qlo in range(8):
            nc.scalar.dma_start(
                out=_ap(e2, qlo * 16 * 32 + qlo, [[32, KHID], [8, 4], [1, 1]]),
                in_=_ap(kw2_bf, hg * 4, [[H, KHID], [1, 4], [1, 1]]))
        E2.append(e2)

    bout_bc = persist.tile([P, DX], F32)
    nc.scalar.dma_start(
        out=bout_bc[:, :],
        in_=bass.AP(tensor=d_bout.tensor, offset=d_bout.offset,
                    ap=[[0, P], [1, DX]]))

    # ================= phase 1: loads, transposes, projections ==============
    p1 = ctx.enter_context(tc.tile_pool(name="p1", bufs=1))
    xh_pool = ctx.enter_context(tc.tile_pool(name="xh", bufs=2))
    w_pool = ctx.enter_context(tc.tile_pool(name="wp", bufs=2))
    xt_pool = ctx.enter_context(tc.tile_pool(name="xt", bufs=2))

    def load_w(dram, pool, name, wait, tag=None):
        if tag is None:
            w = pool.tile([P, NJ, INNER], BF16, name=name)
        else:
            w = pool.tile([P, NJ, INNER], BF16, name=name, tag=tag)
        tc.tile_set_cur_wait(wait)
        nc.gpsimd.dma_start(out=w[:, :, :],
                            in_=dram.rearrange("(t p) i -> p t i", p=P))
        return w

    wk_bf = load_w(d_wk, w_pool, "wk", 0.0002, tag="w")
    wq_bf = load_w(d_wq, w_pool, "wq", 0.0004, tag="w")  # SCALE in qT copies

    def load_x(dram, name, wait):
        """x loaded in 2 half-DMAs (bf16 cast on gpsimd) into transient
        half-tiles, transposed per 128-row tile via xbar into
        xT[p, j, q] = x^T[j*128+p, q]."""
        xT = xt_pool.tile([P, NJ, NQ], BF16, tag="xt", name="xT")
        for hf in range(4):
            tc.tile_set_cur_wait(wait + 0.0008 * hf)
            xb = xh_pool.tile([P, 2, DX], BF16, tag="xh", name="xb")
            nc.gpsimd.dma_start(
                out=xb[:, :, :],
                in_=_ap(dram, hf * 2 * P * DX, [[DX, P], [P * DX, 2], [1, DX]]))
            for tt in range(2):
                t = hf * 2 + tt
                eng = nc.sync if t % 2 == 0 else nc.scalar
                eng.dma_start_transpose(
                    _ap(xT, t * P, [[_fs(xT), P], [NQ, NJ], [1, P]]),
                    xb[:, tt, :])
        return xT

    xkT = load_x(d_xk, "xk", 0.0006)
    xqT = load_x(d_xq, "xq", 0.0040)
    wv_bf = load_w(d_wv, w_pool, "wv", 0.0062, tag="w")
    xvT = load_x(d_xv, "xv", 0.0066)
    wout_bf = load_w(d_wout, persist, "wout", 0.0095)

    qT_bf = persist.tile([P, NJ, NQ], BF16)
    # k^T stored zero-padded per head parity so every dots matmul is a full
    # 128-row stationary at tile_position (0,0): kTe keeps even-head rows
    # (0:64 of each it-slab), kTo keeps odd-head rows; the other half is 0.
    kTe = persist.tile([P, NJ, NKV], BF16)
    kTo = persist.tile([P, NJ, NKV], BF16)
    v5 = persist.tile([P, NKT, H, 65], BF16)
    fs_v5 = _fs(v5)
    tc.tile_set_cur_wait(0.0012)
    nc.vector.memset(kTe[64:128, :, :], 0.0)
    tc.tile_set_cur_wait(0.010)
    nc.gpsimd.memset(kTo[0:64, :, :], 0.0)

    # k-proj: kT[it*128+r, k] = (xk @ wk)[k, it*128+r]  (copies on ACT)
    for it in range(NJ):
        for jk in range(2):
            tc.tile_set_cur_wait(0.0028 + 0.0008 * (it * 2 + jk))
            pk = ps_o.tile([P, 512], F32, tag="po")
            for j in range(NJ):
                nc.tensor.matmul(pk[:, :], wk_bf[:, j, it * P:(it + 1) * P],
                                 xkT[:, j, jk * 512:(jk + 1) * 512],
                                 start=(j == 0), stop=(j == NJ - 1))
            nc.scalar.activation(kTe[0:64, it, jk * 512:(jk + 1) * 512],
                                 pk[0:64, :], AF.Copy)
            nc.scalar.activation(kTo[64:128, it, jk * 512:(jk + 1) * 512],
                                 pk[64:128, :], AF.Copy)

    # q-proj (copies on ACT)
    for it in range(NJ):
        for jq in range(2):
            tc.tile_set_cur_wait(0.0068 + 0.0008 * (it * 2 + jq))
            pq = ps_o.tile([P, 512], F32, tag="po")
            for j in range(NJ):
                nc.tensor.matmul(pq[:, :], wq_bf[:, j, it * P:(it + 1) * P],
                                 xqT[:, j, jq * 512:(jq + 1) * 512],
                                 start=(j == 0), stop=(j == NJ - 1))
            nc.scalar.activation(qT_bf[:, it, jq * 512:(jq + 1) * 512],
                                 pq[:, :], AF.Copy, scale=SCALE)

    # v-proj into v5 [k, kt, h, 64|1]  (copies on DVE)
    for kt in range(NKT):
        tc.tile_set_cur_wait(0.0105 + 0.0008 * kt)
        pv = ps_o.tile([P, 512], F32, tag="po")
        for j in range(NJ):
            nc.tensor.matmul(pv[:, :], xvT[:, j, kt * P:(kt + 1) * P],
                             wv_bf[:, j, :],
                             start=(j == 0), stop=(j == NJ - 1))
        nc.vector.tensor_copy(
            _ap(v5, kt * (H * 65), [[fs_v5, P], [65, H], [1, 64]]),
            pv[:, :])
    nc.gpsimd.memset(
        _ap(v5, 64, [[fs_v5, P], [H * 65, NKT], [65, H], [1, 1]]), 1.0)

    # ================= phase 2: attention main loop =========================
    ps_dt = ctx.enter_context(tc.tile_pool(name="ps_dt", bufs=2, space="PSUM"))
    ps_av = ctx.enter_context(tc.tile_pool(name="ps_av", bufs=2, space="PSUM"))
    r_pool = ctx.enter_context(tc.tile_pool(name="rp", bufs=5))
    pt_pool = ctx.enter_context(tc.tile_pool(name="pt", bufs=2))
    attn_pool = ctx.enter_context(tc.tile_pool(name="attn", bufs=4))
    at_pool = ctx.enter_context(tc.tile_pool(name="at", bufs=1))
    o_pool = ctx.enter_context(tc.tile_pool(name="op", bufs=1))
    zr_pool = ctx.enter_context(tc.tile_pool(name="zr", bufs=2))

    def produce_r(qt, kh, wait, step=0.00019):
        """R~ half-slab for q-tile qt, k-half kh: [P=(qlo,c), qg, k-512]."""
        slab = r_pool.tile([P, 16, KH], BF16, tag="rslab")
        for qg in range(16):
            tc.tile_set_cur_wait(wait + step * qg)
            nc.vector.tensor_scalar(
                out=slab[:, qg, :], in0=B_all[:, kh * KH:(kh + 1) * KH],
                scalar1=a2[:, qt * 16 + qg: qt * 16 + qg + 1],
                scalar2=0.0, op0=AX.add, op1=AX.max)
        return slab

    BASE = 0.0150
    PITCH = 0.0196

    # chunk-0 slabs produced during phase 1 (DVE free after early converts)
    slabs = [[produce_r(0, 0, 0.0020), produce_r(1, 0, 0.0052)],
             [produce_r(0, 1, 0.0084), produce_r(1, 1, 0.0116)]]

    def scores(qc, hg, qc_slabs, t0):
        """8 score groups (one per kt) for head-group hg; exp -> pt."""
        pt = pt_pool.tile([P, 4, NKT, QC], BF16, tag="pt", name="pt")
        fs_pt = _fs(pt)
        for kt in range(NKT):
            with tc.tile_wait_until(t0 + 0.00102 * kt):
                dt = ps_dt.tile([P, 4, QC], F32, tag="dt")
                fs_dt = _fs(dt)
                for bank in range(2):
                    for hh in range(2):
                        h = hg * 4 + bank * 2 + hh
                        it = h // 2
                        kT_h = kTe if h % 2 == 0 else kTo
                        nc.tensor.matmul(
                            dt[:, bank * 2 + hh, :],
                            kT_h[:, it, kt * P:(kt + 1) * P],
                            qT_bf[:, it, qc * QC:(qc + 1) * QC],
                            start=(hh == 0), stop=False,
                            skip_group_check=True)
                    for qth in range(2):
                        slab = qc_slabs[kt // 4][qth]
                        fs_r = _fs(slab)
                        for qg in range(16):
                            last = (qth == 1 and qg == 15)
                            nc.tensor.matmul(
                                _ap(dt, bank * 2 * QC + qth * P + qg,
                                    [[fs_dt, P], [QC, 2], [16, 8]]),
                                _ap(slab, qg * KH + (kt % 4) * P,
                                    [[fs_r, P], [1, P]]),
                                E2[hg][:, bank * 16:(bank + 1) * 16],
                                start=False, stop=last,
                                skip_group_check=True)
                nc.scalar.activation(
                    _ap(pt, kt * QC,
                        [[fs_pt, P], [NKT * QC, 4], [1, QC]]),
                    dt[:, :, :], AF.Exp)
        return pt

    def av_half(qc, hg, pt, attns, t0):
        """AV + normalize for the 4 heads of hg."""
        fs_pt = _fs(pt)
        for i in range(8):
            hh, qt2 = i // 2, i % 2
            h = hg * 4 + hh
            with tc.tile_wait_until(t0 + 0.0002 * i):
                av = ps_av.tile([P, 65], F32, tag="av")
                fs_av = _fs(av)
                for kt in range(NKT):
                    nc.tensor.matmul(
                        av[:, :],
                        _ap(pt, hh * (NKT * QC) + kt * QC + qt2 * P,
                            [[fs_pt, P], [1, P]]),
                        v5[:, kt, h, :],
                        start=(kt == 0), stop=(kt == NKT - 1))
                zr = zr_pool.tile([P, 1], F32, tag="zr")
                nc.vector.reciprocal(
                    zr[:, :], _ap(av, 64, [[fs_av, P], [1, 1]]))
                nc.vector.tensor_scalar(
                    out=attns[qt2][:, h * 64:(h + 1) * 64],
                    in0=_ap(av, 0, [[fs_av, P], [1, 64]]),
                    scalar1=zr[:, :], scalar2=0.0,
                    op0=AX.mult, op1=AX.bypass)

    def tail(qc, attns, t0):
        for qt2 in range(2):
            with tc.tile_wait_until(t0 + 0.0020 * qt2):
                at = at_pool.tile([P, NJ, P], BF16, tag="at")
                eng = nc.sync if qt2 == 0 else nc.scalar
                eng.dma_start_transpose(at[:, :, :], attns[qt2][:, :])
                po = ps_o.tile([P, DX], F32, tag="po")
                for j in range(NJ):
                    nc.tensor.matmul(po[:, :], at[:, j, :], wout_bf[:, j, :],
                                     start=(j == 0), stop=(j == NJ - 1))
                o_sb = o_pool.tile([P, DX], F32, tag="osb")
                nc.vector.tensor_add(o_sb[:, :], po[:, :], bout_bc[:, :])
                qt_g = qc * 2 + qt2
                nc.sync.dma_start(out=d_out[qt_g * P:(qt_g + 1) * P, :],
                                  in_=o_sb[:, :])

    for qc in range(NQC):
        t0 = BASE + PITCH * qc
        qc_slabs = slabs
        if qc + 1 < NQC:
            # k-half-0 slabs of qc+1 go to fresh ring buffers; k-half-1
            # slabs overwrite qc's k-half-0 slabs (free after hg1/kt3)
            slabs = [[produce_r(2 * qc + 2, 0, t0 + 0.0005),
                      produce_r(2 * qc + 3, 0, t0 + 0.0128)],
                     [produce_r(2 * qc + 2, 1, t0 + 0.0160),
                      produce_r(2 * qc + 3, 1, t0 + 0.0192)]]

        a_t0 = attn_pool.tile([P, INNER], BF16, tag="attn", name="a_t0")
        a_t1 = attn_pool.tile([P, INNER], BF16, tag="attn", name="a_t1")
        attns = [a_t0, a_t1]
        pt0 = scores(qc, 0, qc_slabs, t0)
        av_half(qc, 0, pt0, attns, t0 + 0.0090)
        pt1 = scores(qc, 1, qc_slabs, t0 + 0.0083)
        av_half(qc, 1, pt1, attns, t0 + 0.0173)
        tail(qc, attns, t0 + PITCH + 0.0012)

    ctx.close()


_NC_CACHE = None


def _get_nc():
    global _NC_CACHE
    if _NC_CACHE is None:
        nc = bacc.Bacc("TRN2", target_bir_lowering=False, debug=False,
                       enable_asserts=False, num_devices=M)
        build_kernel(nc)
        nc.compile()
        _NC_CACHE = nc
    return _NC_CACHE


def kernel(**inputs):
    nc = _get_nc()
    # kb2 is declared but unused on-device: constant over the softmax axis,
    # it cancels in the softmax.
    shared = {n: np.ascontiguousarray(np.asarray(inputs[n], dtype=np.float32))
              for n in ["w_q", "w_k", "w_v", "w_out", "b_out",
                        "kw1", "kb1", "kw2", "kb2"]}
    in_maps = []
    for i in range(M):
        m = dict(shared)
        for n in ["xq", "xk", "xv", "tq", "tk"]:
            m[n] = np.ascontiguousarray(np.asarray(inputs[n][i], dtype=np.float32))
        in_maps.append(m)
    res = bass_utils.run_bass_kernel_spmd(nc, in_maps, core_ids=list(range(M)))
    out = np.stack([res.results[i]["out"] for i in range(M)], axis=0)
    return out.astype(np.float32)


if __name__ == "__main__":
    import reference
    inputs = {k: np.asarray(v) for k, v in reference.setup_inputs().items()}
    out = kernel(**inputs)
    print("out", out.shape, out.dtype)
